# revision 13
# baseline (speedup 1.0000x reference)
"""MultiHeadSelfAttention (B=4, C=256, H=W=64, 4 heads, GroupNorm32) on 8 trn2 cores.

Sharding: core = (batch b, T-half). The host rolls the T axis so each core's
2048 output tokens are the first TH columns (attention and groupnorm are
order-invariant over s/T, so the roll is transparent). Per core: groupnorm
(channel stats on DVE, group reduce/broadcast via small PE matmuls), qkv
projection, flash-style attention with scores kept transposed [s, t] — exp
output feeds the av matmul directly and the softmax denominator comes from an
appended ones column of v^T — then output projection and residual.

Matmuls run in float32r (rounded fp32, full PE rate at N>=256); everything
else is fp32.
"""

import numpy as np

import concourse.bass as bass
import concourse.mybir as mybir
import concourse.tile as tile
from concourse.bass_utils import run_bass_kernel_spmd

# Problem constants (hardcoded per contract)
B, C, HH, WW = 4, 256, 64, 64
T = HH * WW            # 4096
TH = T // 2            # 2048 tokens per core
NH = 4                 # heads
CH = C // NH           # 64 channels per head
NG = 32                # groupnorm groups
GS = C // NG           # 8 channels per group
EPS = 1e-5
SCALE2 = CH ** -0.5    # 1/8, applied inside exp()
N_CORES = 8

F32 = mybir.dt.float32
F32R = mybir.dt.float32r
AF = mybir.ActivationFunctionType
OP = mybir.AluOpType


def split_excess_waits(nc, max_waits=1):
    """This container's walrus accepts at most one sync-wait condition per
    instruction; move extras onto preceding same-engine NOPs."""
    for f in nc.m.functions:
        for blk in f.blocks:
            new_insts = []
            for inst in blk.instructions:
                si = getattr(inst, "sync_info", None)
                if si is not None and si.on_wait and len(si.on_wait) > max_waits:
                    head = list(si.on_wait)
                    k = 0
                    while len(head) > max_waits:
                        chunk, head = head[:max_waits], head[max_waits:]
                        new_insts.append(mybir.InstNoOp(
                            name=f"{inst.name}-ws{k}", engine=inst.engine,
                            ins=[], outs=[],
                            sync_info=mybir.SyncInfo(on_wait=chunk, on_update=[])))
                        k += 1
                    si.on_wait = head
                new_insts.append(inst)
            blk.instructions = new_insts


def build_nc(repeat=1):
    nc = bass.Bass("TRN2", target_bir_lowering=False, debug=False)

    xb = nc.dram_tensor("xb", [2, 128, T], F32, kind="ExternalInput")
    qkvwt = nc.dram_tensor("qkvwt", [2, 128, 3 * C], F32, kind="ExternalInput")
    projwt = nc.dram_tensor("projwt", [2, 128, C], F32, kind="ExternalInput")
    qkvb = nc.dram_tensor("qkvb", [128, 6], F32, kind="ExternalInput")
    projb = nc.dram_tensor("projb", [128, 2], F32, kind="ExternalInput")
    normw = nc.dram_tensor("normw", [128, 2], F32, kind="ExternalInput")
    normb = nc.dram_tensor("normb", [128, 2], F32, kind="ExternalInput")
    gsum = nc.dram_tensor("gsum", [128, 16], F32, kind="ExternalInput")
    gbc = nc.dram_tensor("gbc", [16, 128], F32, kind="ExternalInput")
    out_d = nc.dram_tensor("out", [2, 128, TH], F32, kind="ExternalOutput")

    import contextlib

    with tile.TileContext(nc) as tc:
        with (
            tc.tile_pool(name="consts", bufs=1) as consts,
            tc.tile_pool(name="xpool", bufs=1) as xpool,
            tc.tile_pool(name="kqv", bufs=1) as kqv,
            tc.For_i(0, repeat, 1) if repeat > 1 else contextlib.nullcontext(),
        ):
            # ---- constant loads ----
            qkvb_sb = consts.tile([128, 6], F32)
            nc.sync.dma_start(out=qkvb_sb, in_=qkvb.ap())
            projb_sb = consts.tile([128, 2], F32)
            nc.sync.dma_start(out=projb_sb, in_=projb.ap())
            normw_sb = consts.tile([128, 2], F32)
            nc.sync.dma_start(out=normw_sb, in_=normw.ap())
            normb_sb = consts.tile([128, 2], F32)
            nc.sync.dma_start(out=normb_sb, in_=normb.ap())
            gsum_sb = consts.tile([128, 16], F32)
            nc.sync.dma_start(out=gsum_sb, in_=gsum.ap())
            gbc_sb = consts.tile([16, 128], F32)
            nc.sync.dma_start(out=gbc_sb, in_=gbc.ap())
            qkvwt_r = consts.tile([128, 2, 3 * C], F32R)
            projwt_r = consts.tile([128, 2, C], F32R)
            scale_sb = consts.tile([128, 2], F32)
            bias_sb = consts.tile([128, 2], F32)

            # x: [128, chunk, T]; first TH columns are this core's tokens
            x_sb = xpool.tile([128, 2, T], F32)
            for k in range(2):
                nc.sync.dma_start(out=x_sb[:, k, :], in_=xb.ap()[k])

            # persistent qkv outputs
            k_sb = kqv.tile([128, 2, T], F32R)
            q_sb = kqv.tile([128, 2, TH], F32R)
            vt_sb = kqv.tile([128, T // 128, NH, CH + 1], F32R)
            ones_st = consts.tile([128, T // 128, 1], F32)
            nc.vector.memset(ones_st, 1.0)
            for h in range(NH):
                nc.vector.tensor_copy(out=vt_sb[:, :, h, CH:CH + 1], in_=ones_st)
            ones_row_f = consts.tile([1, CH], F32)
            nc.vector.memset(ones_row_f, 1.0)
            ones_row = consts.tile([1, CH], F32R)
            nc.vector.tensor_copy(out=ones_row, in_=ones_row_f)

            with (
                tc.tile_pool(name="stage", bufs=1) as stage,
                tc.tile_pool(name="psmall", bufs=1, space="PSUM") as psmall,
            ):
                qkvwt_f = stage.tile([128, 2, 3 * C], F32)
                projwt_f = stage.tile([128, 2, C], F32)
                for k in range(2):
                    nc.sync.dma_start(out=qkvwt_f[:, k, :], in_=qkvwt.ap()[k])
                    nc.sync.dma_start(out=projwt_f[:, k, :], in_=projwt.ap()[k])
                nc.vector.tensor_copy(out=qkvwt_r, in_=qkvwt_f)
                nc.vector.tensor_copy(out=projwt_r, in_=projwt_f)

                # ---- groupnorm statistics ----
                # per-channel (mean, m2) -> per-group via PE (gsum), rstd,
                # then broadcast groups back to channels via PE (gbc).
                stat = stage.tile([128, 2, 2], F32)  # (mean_c, m2_c) per chunk
                sq = stage.tile([128, 1], F32)
                for k in range(2):
                    st6 = stage.tile([128, 8, 6], F32, bufs=2)
                    for sub in range(8):
                        nc.vector.bn_stats(out=st6[:, sub, :],
                                           in_=x_sb[:, k, 512 * sub:512 * (sub + 1)])
                    nc.vector.bn_aggr(out=stat[:, k, :], in_=st6)
                    # m2 = var + mean^2
                    nc.vector.tensor_tensor(out=sq, in0=stat[:, k, 0:1],
                                            in1=stat[:, k, 0:1], op=OP.mult)
                    nc.vector.tensor_tensor(out=stat[:, k, 1:2], in0=stat[:, k, 1:2],
                                            in1=sq, op=OP.add)
                pgrp = psmall.tile([16, 4], F32)
                nc.tensor.matmul(pgrp, gsum_sb,
                                 stat.rearrange("p a b -> p (a b)"),
                                 start=True, stop=True)
                # pgrp columns: (chunk k, val v) at 2k+v; v=0 mean, v=1 m2
                pgrp_kv = pgrp.rearrange("g (k v) -> g v k", v=2)
                meang = stage.tile([16, 2], F32)
                nc.vector.tensor_copy(out=meang, in_=pgrp_kv[:, 0, :])
                sqg = stage.tile([16, 2], F32)
                nc.vector.tensor_tensor(out=sqg, in0=meang, in1=meang, op=OP.mult)
                rstdg = stage.tile([16, 2], F32)
                nc.vector.tensor_tensor(out=rstdg, in0=pgrp_kv[:, 1, :], in1=sqg,
                                        op=OP.subtract)
                eps_t = stage.tile([16, 1], F32)
                nc.vector.memset(eps_t, EPS)
                nc.scalar.activation(out=rstdg, in_=rstdg, func=AF.Sqrt, bias=eps_t)
                nc.vector.reciprocal(out=rstdg, in_=rstdg)
                pm = psmall.tile([128, 2], F32)
                nc.tensor.matmul(pm, gbc_sb, meang, start=True, stop=True)
                pr = psmall.tile([128, 2], F32)
                nc.tensor.matmul(pr, gbc_sb, rstdg, start=True, stop=True)
                nc.vector.tensor_tensor(out=scale_sb, in0=pr, in1=normw_sb,
                                        op=OP.mult)
                nc.vector.tensor_tensor(out=bias_sb, in0=pm, in1=scale_sb,
                                        op=OP.mult)
                nc.vector.tensor_tensor(out=bias_sb, in0=normb_sb, in1=bias_sb,
                                        op=OP.subtract)

            # ---- qkv projection, streamed over T in 512-col tiles ----
            with (
                tc.tile_pool(name="xn", bufs=3) as xnp,
                tc.tile_pool(name="psqk", bufs=3, space="PSUM") as psqk,
                tc.tile_pool(name="psv", bufs=3, space="PSUM") as psv,
            ):
                for t8 in range(8):
                    t0 = 512 * t8
                    xn_t = xnp.tile([128, 2, 512], F32R)
                    for k in range(2):
                        nc.vector.tensor_scalar(
                            out=xn_t[:, k, :], in0=x_sb[:, k, t0:t0 + 512],
                            scalar1=scale_sb[:, k:k + 1],
                            scalar2=bias_sb[:, k:k + 1],
                            op0=OP.mult, op1=OP.add)
                    # k tiles (full T), q tiles (first TH only)
                    for j in range(2):
                        pk = psqk.tile([128, 512], F32, tag="qk")
                        for k in range(2):
                            nc.tensor.matmul(
                                pk, qkvwt_r[:, k, C + 128 * j:C + 128 * (j + 1)],
                                xn_t[:, k, :], start=(k == 0), stop=(k == 1))
                        nc.vector.tensor_scalar_add(
                            out=k_sb[:, j, t0:t0 + 512], in0=pk,
                            scalar1=qkvb_sb[:, 2 + j:3 + j])
                        if t8 < 4:
                            pq = psqk.tile([128, 512], F32, tag="qk")
                            for k in range(2):
                                nc.tensor.matmul(
                                    pq, qkvwt_r[:, k, 128 * j:128 * (j + 1)],
                                    xn_t[:, k, :], start=(k == 0), stop=(k == 1))
                            nc.vector.tensor_scalar_add(
                                out=q_sb[:, j, t0:t0 + 512], in0=pq,
                                scalar1=qkvb_sb[:, j:j + 1])
                    # v^T tiles
                    for u in range(4):
                        pv = psv.tile([128, C], F32)
                        for k in range(2):
                            nc.tensor.matmul(
                                pv, xn_t[:, k, 128 * u:128 * (u + 1)],
                                qkvwt_r[:, k, 2 * C:3 * C],
                                start=(k == 0), stop=(k == 1))
                        nc.vector.tensor_copy(
                            out=vt_sb[:, 4 * t8 + u, :, 0:CH], in_=pv)

            # ---- attention + projection ----
            with (
                tc.tile_pool(name="apool", bufs=1) as apool,
                tc.tile_pool(name="wexp", bufs=6) as wexp,
                tc.tile_pool(name="rpool", bufs=2) as rpool,
                tc.tile_pool(name="opool", bufs=1) as opool,
                tc.tile_pool(name="pss", bufs=2, space="PSUM") as pss,
                tc.tile_pool(name="psa", bufs=2, space="PSUM") as psa,
                tc.tile_pool(name="psh", bufs=2, space="PSUM") as psh,
            ):
                a_sb = apool.tile([128, 2, TH], F32R)
                out_sb = opool.tile([128, 2, TH], F32)
                n_p = T // 256  # 16 paired score chunks (2 x 128 rows each)
                for ti in range(4):
                    t0 = 512 * ti
                    for h in range(NH):
                        p0 = CH * (h % 2)
                        j = h // 2
                        pa = psa.tile([CH + 1, 512], F32)
                        # software-pipelined: scores(i+1) issued before av(i)
                        w_l = [None, None]
                        for s in range(n_p + 1):
                            if s < n_p:
                                # scores for chunks 2s, 2s+1 -> one psum pair
                                ps = pss.tile([128, 2, 512], F32)
                                for u in range(2):
                                    nc.tensor.matmul(
                                        ps[:, u, :],
                                        k_sb[p0:p0 + CH, j,
                                             128 * (2 * s + u):128 * (2 * s + u + 1)],
                                        q_sb[p0:p0 + CH, j, t0:t0 + 512],
                                        start=True, stop=True)
                                w_t = wexp.tile([128, 2, 512], F32R)
                                nc.scalar.activation(out=w_t, in_=ps, func=AF.Exp,
                                                     scale=SCALE2)
                                w_l[s % 2] = w_t
                            if s > 0:
                                for u in range(2):
                                    sc = 2 * (s - 1) + u
                                    nc.tensor.matmul(
                                        pa, vt_sb[:, sc, h, :],
                                        w_l[(s - 1) % 2][:, u, :],
                                        start=(sc == 0), stop=(sc == 2 * n_p - 1))
                        recip = rpool.tile([1, 512], F32)
                        nc.vector.reciprocal(out=recip, in_=pa[CH:CH + 1, :])
                        recip_r = rpool.tile([1, 512], F32R)
                        nc.vector.tensor_copy(out=recip_r, in_=recip)
                        rb = psh.tile([128, 512], F32, tag="ph")
                        nc.tensor.matmul(rb[0:CH, :], ones_row, recip_r,
                                         start=True, stop=True)
                        a_sl = a_sb[p0:p0 + CH, j, t0:t0 + 512]
                        nc.vector.tensor_copy(out=a_sl, in_=pa[0:CH, :])
                        nc.vector.tensor_tensor(out=a_sl, in0=a_sl,
                                                in1=rb[0:CH, :], op=OP.mult)
                        nc.vector.tensor_scalar_add(
                            out=a_sl, in0=a_sl,
                            scalar1=qkvb_sb[p0:p0 + CH, 4 + j:5 + j])
                    # proj for this t-tile (all heads of column range ready)
                    for j in range(2):
                        ph = psh.tile([128, 512], F32)
                        for k in range(2):
                            nc.tensor.matmul(
                                ph, projwt_r[:, k, 128 * j:128 * (j + 1)],
                                a_sb[:, k, t0:t0 + 512], start=(k == 0),
                                stop=(k == 1))
                        o_sl = out_sb[:, j, t0:t0 + 512]
                        nc.vector.tensor_tensor(out=o_sl, in0=ph,
                                                in1=x_sb[:, j, t0:t0 + 512],
                                                op=OP.add)
                        nc.vector.tensor_scalar_add(
                            out=o_sl, in0=o_sl, scalar1=projb_sb[:, j:j + 1])
                        nc.sync.dma_start(out=out_d.ap()[j, :, t0:t0 + 512],
                                          in_=o_sl)

    split_excess_waits(nc)
    return nc


_NC_CACHE = {}


def _get_nc(repeat=1):
    if repeat not in _NC_CACHE:
        _NC_CACHE[repeat] = build_nc(repeat)
    return _NC_CACHE[repeat]


def _shard_inputs(x, norm_w, norm_b, qkv_w, qkv_b, proj_w, proj_b):
    xr = np.ascontiguousarray(x.reshape(B, 2, 128, T).astype(np.float32))
    # Reference splits qkv head-blockwise: head h uses rows [192h, 192h+192)
    # as (q|k|v). Permute rows to our layout: q all heads head-major, then k,
    # then v.
    perm = np.concatenate([
        np.concatenate([np.arange(3 * CH * h + CH * p, 3 * CH * h + CH * (p + 1))
                        for h in range(NH)])
        for p in range(3)])
    qkv_w = np.asarray(qkv_w)[perm]
    qkv_b = np.asarray(qkv_b)[perm]
    qkvwt = np.ascontiguousarray(qkv_w.T.reshape(2, 128, 3 * C).astype(np.float32))
    projwt = np.ascontiguousarray(proj_w.T.reshape(2, 128, C).astype(np.float32))
    qkvb = np.ascontiguousarray(qkv_b.reshape(6, 128).T.astype(np.float32))
    projb = np.ascontiguousarray(proj_b.reshape(2, 128).T.astype(np.float32))
    normw = np.ascontiguousarray(norm_w.reshape(2, 128).T.astype(np.float32))
    normb = np.ascontiguousarray(norm_b.reshape(2, 128).T.astype(np.float32))
    p = np.arange(128)
    gsum = (p[:, None] // 8 == np.arange(16)[None, :]).astype(np.float32) / GS
    gbc = (np.arange(16)[:, None] == p[None, :] // 8).astype(np.float32)

    in_maps = []
    for c in range(N_CORES):
        b, half = c // 2, c % 2
        # roll T so this core's tokens are the first TH columns
        xc = np.roll(xr[b], -half * TH, axis=2) if half else xr[b]
        in_maps.append({
            "xb": np.ascontiguousarray(xc),
            "qkvwt": qkvwt, "projwt": projwt,
            "qkvb": qkvb, "projb": projb,
            "normw": normw, "normb": normb,
            "gsum": gsum, "gbc": gbc,
        })
    return in_maps


def _assemble(results):
    out = np.empty((B, 2, 128, T), np.float32)
    for c in range(N_CORES):
        b, half = c // 2, c % 2
        out[b, :, :, half * TH:(half + 1) * TH] = results[c]["out"]
    return out.reshape(B, C, HH, WW)


def kernel(x, norm_w, norm_b, qkv_w, qkv_b, proj_w, proj_b):
    nc = _get_nc()
    in_maps = _shard_inputs(x, norm_w, norm_b, qkv_w, qkv_b, proj_w, proj_b)
    res = run_bass_kernel_spmd(nc, in_maps, core_ids=list(range(N_CORES)))
    return _assemble(res.results)


# revision 14
# speedup vs baseline: 1.0863x; 1.0863x over previous
"""MultiHeadSelfAttention (B=4, C=256, H=W=64, 4 heads, GroupNorm32) on 8 trn2 cores.

Sharding: core = (batch b, T-half). The host rolls the T axis so each core's
2048 output tokens are the first TH columns (attention and groupnorm are
order-invariant over s/T, so the roll is transparent). Per core: groupnorm
(channel stats on DVE, group reduce/broadcast via small PE matmuls), qkv
projection, flash-style attention with scores kept transposed [s, t] — exp
output feeds the av matmul directly and the softmax denominator comes from an
appended ones column of v^T — then output projection and residual.

Matmuls run in float32r (rounded fp32, full PE rate at N>=256); everything
else is fp32.
"""

import numpy as np

import concourse.bass as bass
import concourse.mybir as mybir
import concourse.tile as tile
from concourse.bass_utils import run_bass_kernel_spmd

# Problem constants (hardcoded per contract)
B, C, HH, WW = 4, 256, 64, 64
T = HH * WW            # 4096
TH = T // 2            # 2048 tokens per core
NH = 4                 # heads
CH = C // NH           # 64 channels per head
NG = 32                # groupnorm groups
GS = C // NG           # 8 channels per group
EPS = 1e-5
SCALE2 = CH ** -0.5    # 1/8, applied inside exp()
N_CORES = 8

F32 = mybir.dt.float32
F32R = mybir.dt.float32r
AF = mybir.ActivationFunctionType
OP = mybir.AluOpType


def split_excess_waits(nc, max_waits=1):
    """This container's walrus accepts at most one sync-wait condition per
    instruction; move extras onto preceding same-engine NOPs."""
    for f in nc.m.functions:
        for blk in f.blocks:
            new_insts = []
            for inst in blk.instructions:
                si = getattr(inst, "sync_info", None)
                if si is not None and si.on_wait and len(si.on_wait) > max_waits:
                    head = list(si.on_wait)
                    k = 0
                    while len(head) > max_waits:
                        chunk, head = head[:max_waits], head[max_waits:]
                        new_insts.append(mybir.InstNoOp(
                            name=f"{inst.name}-ws{k}", engine=inst.engine,
                            ins=[], outs=[],
                            sync_info=mybir.SyncInfo(on_wait=chunk, on_update=[])))
                        k += 1
                    si.on_wait = head
                new_insts.append(inst)
            blk.instructions = new_insts


def build_nc(repeat=1):
    nc = bass.Bass("TRN2", target_bir_lowering=False, debug=False)

    xb = nc.dram_tensor("xb", [2, 128, T], F32, kind="ExternalInput")
    qkvwt = nc.dram_tensor("qkvwt", [2, 128, 3 * C], F32, kind="ExternalInput")
    projwt = nc.dram_tensor("projwt", [2, 128, C], F32, kind="ExternalInput")
    qkvb = nc.dram_tensor("qkvb", [128, 6], F32, kind="ExternalInput")
    projb = nc.dram_tensor("projb", [128, 2], F32, kind="ExternalInput")
    normw = nc.dram_tensor("normw", [128, 2], F32, kind="ExternalInput")
    normb = nc.dram_tensor("normb", [128, 2], F32, kind="ExternalInput")
    gsum = nc.dram_tensor("gsum", [128, 16], F32, kind="ExternalInput")
    gbc = nc.dram_tensor("gbc", [16, 128], F32, kind="ExternalInput")
    out_d = nc.dram_tensor("out", [2, 128, TH], F32, kind="ExternalOutput")

    import contextlib

    with tile.TileContext(nc) as tc:
        with (
            tc.tile_pool(name="consts", bufs=1) as consts,
            tc.tile_pool(name="xpool", bufs=1) as xpool,
            tc.tile_pool(name="kqv", bufs=1) as kqv,
            tc.For_i(0, repeat, 1) if repeat > 1 else contextlib.nullcontext(),
        ):
            # ---- constant loads ----
            qkvb_sb = consts.tile([128, 6], F32)
            nc.sync.dma_start(out=qkvb_sb, in_=qkvb.ap())
            projb_sb = consts.tile([128, 2], F32)
            nc.sync.dma_start(out=projb_sb, in_=projb.ap())
            normw_sb = consts.tile([128, 2], F32)
            nc.sync.dma_start(out=normw_sb, in_=normw.ap())
            normb_sb = consts.tile([128, 2], F32)
            nc.sync.dma_start(out=normb_sb, in_=normb.ap())
            gsum_sb = consts.tile([128, 16], F32)
            nc.sync.dma_start(out=gsum_sb, in_=gsum.ap())
            gbc_sb = consts.tile([16, 128], F32)
            nc.sync.dma_start(out=gbc_sb, in_=gbc.ap())
            qkvwt_r = consts.tile([128, 2, 3 * C], F32R)
            projwt_r = consts.tile([128, 2, C], F32R)
            scale_sb = consts.tile([128, 2], F32)
            bias_sb = consts.tile([128, 2], F32)

            # x: [128, chunk, T]; first TH columns are this core's tokens
            x_sb = xpool.tile([128, 2, T], F32)
            for k in range(2):
                for q4 in range(4):
                    c0 = 1024 * q4
                    nc.sync.dma_start(out=x_sb[:, k, c0:c0 + 1024],
                                      in_=xb.ap()[k][:, c0:c0 + 1024])

            # persistent qkv outputs
            k_sb = kqv.tile([128, 2, T], F32R)
            q_sb = kqv.tile([128, 2, TH], F32R)
            vt_sb = kqv.tile([128, T // 128, NH, CH + 1], F32R)
            ones_st = consts.tile([128, T // 128, 1], F32)
            nc.vector.memset(ones_st, 1.0)
            for h in range(NH):
                nc.vector.tensor_copy(out=vt_sb[:, :, h, CH:CH + 1], in_=ones_st)
            ones_row_f = consts.tile([1, CH], F32)
            nc.vector.memset(ones_row_f, 1.0)
            ones_row = consts.tile([1, CH], F32R)
            nc.vector.tensor_copy(out=ones_row, in_=ones_row_f)

            with (
                tc.tile_pool(name="stage", bufs=1) as stage,
                tc.tile_pool(name="psmall", bufs=1, space="PSUM") as psmall,
            ):
                qkvwt_f = stage.tile([128, 2, 3 * C], F32)
                projwt_f = stage.tile([128, 2, C], F32)
                for k in range(2):
                    nc.sync.dma_start(out=qkvwt_f[:, k, :], in_=qkvwt.ap()[k])
                    nc.sync.dma_start(out=projwt_f[:, k, :], in_=projwt.ap()[k])
                nc.vector.tensor_copy(out=qkvwt_r, in_=qkvwt_f)
                nc.vector.tensor_copy(out=projwt_r, in_=projwt_f)

                # ---- groupnorm statistics ----
                # per-channel (mean, m2) -> per-group via PE (gsum), rstd,
                # then broadcast groups back to channels via PE (gbc).
                stat = stage.tile([128, 2, 2], F32)  # (mean_c, m2_c) per chunk
                sq = stage.tile([128, 1], F32)
                for k in range(2):
                    st6 = stage.tile([128, 8, 6], F32, bufs=2)
                    for sub in range(8):
                        nc.vector.bn_stats(out=st6[:, sub, :],
                                           in_=x_sb[:, k, 512 * sub:512 * (sub + 1)])
                    nc.vector.bn_aggr(out=stat[:, k, :], in_=st6)
                    # m2 = var + mean^2
                    nc.vector.tensor_tensor(out=sq, in0=stat[:, k, 0:1],
                                            in1=stat[:, k, 0:1], op=OP.mult)
                    nc.vector.tensor_tensor(out=stat[:, k, 1:2], in0=stat[:, k, 1:2],
                                            in1=sq, op=OP.add)
                pgrp = psmall.tile([16, 4], F32)
                nc.tensor.matmul(pgrp, gsum_sb,
                                 stat.rearrange("p a b -> p (a b)"),
                                 start=True, stop=True)
                # pgrp columns: (chunk k, val v) at 2k+v; v=0 mean, v=1 m2
                pgrp_kv = pgrp.rearrange("g (k v) -> g v k", v=2)
                meang = stage.tile([16, 2], F32)
                nc.vector.tensor_copy(out=meang, in_=pgrp_kv[:, 0, :])
                sqg = stage.tile([16, 2], F32)
                nc.vector.tensor_tensor(out=sqg, in0=meang, in1=meang, op=OP.mult)
                rstdg = stage.tile([16, 2], F32)
                nc.vector.tensor_tensor(out=rstdg, in0=pgrp_kv[:, 1, :], in1=sqg,
                                        op=OP.subtract)
                eps_t = stage.tile([16, 1], F32)
                nc.vector.memset(eps_t, EPS)
                nc.scalar.activation(out=rstdg, in_=rstdg, func=AF.Sqrt, bias=eps_t)
                nc.vector.reciprocal(out=rstdg, in_=rstdg)
                pm = psmall.tile([128, 2], F32)
                nc.tensor.matmul(pm, gbc_sb, meang, start=True, stop=True)
                pr = psmall.tile([128, 2], F32)
                nc.tensor.matmul(pr, gbc_sb, rstdg, start=True, stop=True)
                nc.vector.tensor_tensor(out=scale_sb, in0=pr, in1=normw_sb,
                                        op=OP.mult)
                nc.vector.tensor_tensor(out=bias_sb, in0=pm, in1=scale_sb,
                                        op=OP.mult)
                nc.vector.tensor_tensor(out=bias_sb, in0=normb_sb, in1=bias_sb,
                                        op=OP.subtract)

            # ---- qkv projection, streamed over T in 512-col tiles ----
            with (
                tc.tile_pool(name="xn", bufs=3) as xnp,
                tc.tile_pool(name="psqk", bufs=3, space="PSUM") as psqk,
                tc.tile_pool(name="psv", bufs=3, space="PSUM") as psv,
            ):
                for t8 in range(8):
                    t0 = 512 * t8
                    xn_t = xnp.tile([128, 2, 512], F32R)
                    for k in range(2):
                        nc.vector.tensor_scalar(
                            out=xn_t[:, k, :], in0=x_sb[:, k, t0:t0 + 512],
                            scalar1=scale_sb[:, k:k + 1],
                            scalar2=bias_sb[:, k:k + 1],
                            op0=OP.mult, op1=OP.add)
                    # k tiles (full T), q tiles (first TH only)
                    for j in range(2):
                        pk = psqk.tile([128, 512], F32, tag="qk")
                        for k in range(2):
                            nc.tensor.matmul(
                                pk, qkvwt_r[:, k, C + 128 * j:C + 128 * (j + 1)],
                                xn_t[:, k, :], start=(k == 0), stop=(k == 1))
                        nc.vector.tensor_scalar_add(
                            out=k_sb[:, j, t0:t0 + 512], in0=pk,
                            scalar1=qkvb_sb[:, 2 + j:3 + j])
                        if t8 < 4:
                            pq = psqk.tile([128, 512], F32, tag="qk")
                            for k in range(2):
                                nc.tensor.matmul(
                                    pq, qkvwt_r[:, k, 128 * j:128 * (j + 1)],
                                    xn_t[:, k, :], start=(k == 0), stop=(k == 1))
                            nc.vector.tensor_scalar_add(
                                out=q_sb[:, j, t0:t0 + 512], in0=pq,
                                scalar1=qkvb_sb[:, j:j + 1])
                    # v^T tiles
                    for u in range(4):
                        pv = psv.tile([128, C], F32)
                        for k in range(2):
                            nc.tensor.matmul(
                                pv, xn_t[:, k, 128 * u:128 * (u + 1)],
                                qkvwt_r[:, k, 2 * C:3 * C],
                                start=(k == 0), stop=(k == 1))
                        nc.vector.tensor_copy(
                            out=vt_sb[:, 4 * t8 + u, :, 0:CH], in_=pv)

            # ---- attention + projection ----
            with (
                tc.tile_pool(name="apool", bufs=1) as apool,
                tc.tile_pool(name="wexp", bufs=6) as wexp,
                tc.tile_pool(name="rpool", bufs=2) as rpool,
                tc.tile_pool(name="opool", bufs=1) as opool,
                tc.tile_pool(name="pss", bufs=2, space="PSUM") as pss,
                tc.tile_pool(name="psa", bufs=2, space="PSUM") as psa,
                tc.tile_pool(name="psh", bufs=2, space="PSUM") as psh,
            ):
                a_sb = apool.tile([128, 2, TH], F32R)
                out_sb = opool.tile([128, 2, TH], F32)
                n_p = T // 256  # 16 paired score chunks (2 x 128 rows each)
                for ti in range(4):
                    t0 = 512 * ti
                    for h in range(NH):
                        p0 = CH * (h % 2)
                        j = h // 2
                        pa = psa.tile([CH + 1, 512], F32)
                        # software-pipelined: scores(i+1) issued before av(i)
                        w_l = [None, None]
                        for s in range(n_p + 1):
                            if s < n_p:
                                # scores for chunks 2s, 2s+1 -> one psum pair
                                ps = pss.tile([128, 2, 512], F32)
                                for u in range(2):
                                    nc.tensor.matmul(
                                        ps[:, u, :],
                                        k_sb[p0:p0 + CH, j,
                                             128 * (2 * s + u):128 * (2 * s + u + 1)],
                                        q_sb[p0:p0 + CH, j, t0:t0 + 512],
                                        start=True, stop=True)
                                w_t = wexp.tile([128, 2, 512], F32R)
                                nc.scalar.activation(out=w_t, in_=ps, func=AF.Exp,
                                                     scale=SCALE2)
                                w_l[s % 2] = w_t
                            if s > 0:
                                for u in range(2):
                                    sc = 2 * (s - 1) + u
                                    nc.tensor.matmul(
                                        pa, vt_sb[:, sc, h, :],
                                        w_l[(s - 1) % 2][:, u, :],
                                        start=(sc == 0), stop=(sc == 2 * n_p - 1))
                        recip = rpool.tile([1, 512], F32)
                        nc.vector.reciprocal(out=recip, in_=pa[CH:CH + 1, :])
                        recip_r = rpool.tile([1, 512], F32R)
                        nc.vector.tensor_copy(out=recip_r, in_=recip)
                        rb = psh.tile([128, 512], F32, tag="ph")
                        nc.tensor.matmul(rb[0:CH, :], ones_row, recip_r,
                                         start=True, stop=True)
                        a_sl = a_sb[p0:p0 + CH, j, t0:t0 + 512]
                        nc.vector.tensor_copy(out=a_sl, in_=pa[0:CH, :])
                        nc.vector.tensor_tensor(out=a_sl, in0=a_sl,
                                                in1=rb[0:CH, :], op=OP.mult)
                        nc.vector.tensor_scalar_add(
                            out=a_sl, in0=a_sl,
                            scalar1=qkvb_sb[p0:p0 + CH, 4 + j:5 + j])
                    # proj for this t-tile (all heads of column range ready)
                    for j in range(2):
                        ph = psh.tile([128, 512], F32)
                        for k in range(2):
                            nc.tensor.matmul(
                                ph, projwt_r[:, k, 128 * j:128 * (j + 1)],
                                a_sb[:, k, t0:t0 + 512], start=(k == 0),
                                stop=(k == 1))
                        o_sl = out_sb[:, j, t0:t0 + 512]
                        nc.vector.tensor_tensor(out=o_sl, in0=ph,
                                                in1=x_sb[:, j, t0:t0 + 512],
                                                op=OP.add)
                        nc.vector.tensor_scalar_add(
                            out=o_sl, in0=o_sl, scalar1=projb_sb[:, j:j + 1])
                        nc.sync.dma_start(out=out_d.ap()[j, :, t0:t0 + 512],
                                          in_=o_sl)

    split_excess_waits(nc)
    return nc


_NC_CACHE = {}


def _get_nc(repeat=1):
    if repeat not in _NC_CACHE:
        _NC_CACHE[repeat] = build_nc(repeat)
    return _NC_CACHE[repeat]


def _shard_inputs(x, norm_w, norm_b, qkv_w, qkv_b, proj_w, proj_b):
    xr = np.ascontiguousarray(x.reshape(B, 2, 128, T).astype(np.float32))
    # Reference splits qkv head-blockwise: head h uses rows [192h, 192h+192)
    # as (q|k|v). Permute rows to our layout: q all heads head-major, then k,
    # then v.
    perm = np.concatenate([
        np.concatenate([np.arange(3 * CH * h + CH * p, 3 * CH * h + CH * (p + 1))
                        for h in range(NH)])
        for p in range(3)])
    qkv_w = np.asarray(qkv_w)[perm]
    qkv_b = np.asarray(qkv_b)[perm]
    qkvwt = np.ascontiguousarray(qkv_w.T.reshape(2, 128, 3 * C).astype(np.float32))
    projwt = np.ascontiguousarray(proj_w.T.reshape(2, 128, C).astype(np.float32))
    qkvb = np.ascontiguousarray(qkv_b.reshape(6, 128).T.astype(np.float32))
    projb = np.ascontiguousarray(proj_b.reshape(2, 128).T.astype(np.float32))
    normw = np.ascontiguousarray(norm_w.reshape(2, 128).T.astype(np.float32))
    normb = np.ascontiguousarray(norm_b.reshape(2, 128).T.astype(np.float32))
    p = np.arange(128)
    gsum = (p[:, None] // 8 == np.arange(16)[None, :]).astype(np.float32) / GS
    gbc = (np.arange(16)[:, None] == p[None, :] // 8).astype(np.float32)

    in_maps = []
    for c in range(N_CORES):
        b, half = c // 2, c % 2
        # roll T so this core's tokens are the first TH columns
        xc = np.roll(xr[b], -half * TH, axis=2) if half else xr[b]
        in_maps.append({
            "xb": np.ascontiguousarray(xc),
            "qkvwt": qkvwt, "projwt": projwt,
            "qkvb": qkvb, "projb": projb,
            "normw": normw, "normb": normb,
            "gsum": gsum, "gbc": gbc,
        })
    return in_maps


def _assemble(results):
    out = np.empty((B, 2, 128, T), np.float32)
    for c in range(N_CORES):
        b, half = c // 2, c % 2
        out[b, :, :, half * TH:(half + 1) * TH] = results[c]["out"]
    return out.reshape(B, C, HH, WW)


def kernel(x, norm_w, norm_b, qkv_w, qkv_b, proj_w, proj_b):
    nc = _get_nc()
    in_maps = _shard_inputs(x, norm_w, norm_b, qkv_w, qkv_b, proj_w, proj_b)
    res = run_bass_kernel_spmd(nc, in_maps, core_ids=list(range(N_CORES)))
    return _assemble(res.results)


# revision 15
# speedup vs baseline: 1.0993x; 1.0119x over previous
"""MultiHeadSelfAttention (B=4, C=256, H=W=64, 4 heads, GroupNorm32) on 8 trn2 cores.

Sharding: core = (batch b, T-half). The host rolls the T axis so each core's
2048 output tokens are the first TH columns (attention and groupnorm are
order-invariant over s/T, so the roll is transparent). Per core: groupnorm
(channel stats on DVE, group reduce/broadcast via small PE matmuls), qkv
projection, flash-style attention with scores kept transposed [s, t] — exp
output feeds the av matmul directly and the softmax denominator comes from an
appended ones column of v^T — then output projection and residual.

Matmuls run in float32r (rounded fp32, full PE rate at N>=256); everything
else is fp32.
"""

import numpy as np

import concourse.bass as bass
import concourse.mybir as mybir
import concourse.tile as tile
from concourse.bass_utils import run_bass_kernel_spmd

# Problem constants (hardcoded per contract)
B, C, HH, WW = 4, 256, 64, 64
T = HH * WW            # 4096
TH = T // 2            # 2048 tokens per core
NH = 4                 # heads
CH = C // NH           # 64 channels per head
NG = 32                # groupnorm groups
GS = C // NG           # 8 channels per group
EPS = 1e-5
SCALE2 = CH ** -0.5    # 1/8, applied inside exp()
N_CORES = 8

F32 = mybir.dt.float32
F32R = mybir.dt.float32r
AF = mybir.ActivationFunctionType
OP = mybir.AluOpType


def split_excess_waits(nc, max_waits=1):
    """This container's walrus accepts at most one sync-wait condition per
    instruction; move extras onto preceding same-engine NOPs."""
    for f in nc.m.functions:
        for blk in f.blocks:
            new_insts = []
            for inst in blk.instructions:
                si = getattr(inst, "sync_info", None)
                if si is not None and si.on_wait and len(si.on_wait) > max_waits:
                    head = list(si.on_wait)
                    k = 0
                    while len(head) > max_waits:
                        chunk, head = head[:max_waits], head[max_waits:]
                        new_insts.append(mybir.InstNoOp(
                            name=f"{inst.name}-ws{k}", engine=inst.engine,
                            ins=[], outs=[],
                            sync_info=mybir.SyncInfo(on_wait=chunk, on_update=[])))
                        k += 1
                    si.on_wait = head
                new_insts.append(inst)
            blk.instructions = new_insts


def build_nc(repeat=1):
    nc = bass.Bass("TRN2", target_bir_lowering=False, debug=False)

    xb = nc.dram_tensor("xb", [2, 128, T], F32, kind="ExternalInput")
    qkvwt = nc.dram_tensor("qkvwt", [2, 128, 3 * C], F32, kind="ExternalInput")
    projwt = nc.dram_tensor("projwt", [2, 128, C], F32, kind="ExternalInput")
    qkvb = nc.dram_tensor("qkvb", [128, 6], F32, kind="ExternalInput")
    projb = nc.dram_tensor("projb", [128, 2], F32, kind="ExternalInput")
    normw = nc.dram_tensor("normw", [128, 2], F32, kind="ExternalInput")
    normb = nc.dram_tensor("normb", [128, 2], F32, kind="ExternalInput")
    gsum = nc.dram_tensor("gsum", [128, 16], F32, kind="ExternalInput")
    gbc = nc.dram_tensor("gbc", [16, 128], F32, kind="ExternalInput")
    out_d = nc.dram_tensor("out", [2, 128, TH], F32, kind="ExternalOutput")

    import contextlib

    with tile.TileContext(nc) as tc:
        with (
            tc.tile_pool(name="consts", bufs=1) as consts,
            tc.tile_pool(name="xpool", bufs=1) as xpool,
            tc.tile_pool(name="kqv", bufs=1) as kqv,
            tc.For_i(0, repeat, 1) if repeat > 1 else contextlib.nullcontext(),
        ):
            # ---- constant loads ----
            qkvb_sb = consts.tile([128, 6], F32)
            nc.sync.dma_start(out=qkvb_sb, in_=qkvb.ap())
            projb_sb = consts.tile([128, 2], F32)
            nc.sync.dma_start(out=projb_sb, in_=projb.ap())
            normw_sb = consts.tile([128, 2], F32)
            nc.sync.dma_start(out=normw_sb, in_=normw.ap())
            normb_sb = consts.tile([128, 2], F32)
            nc.sync.dma_start(out=normb_sb, in_=normb.ap())
            gsum_sb = consts.tile([128, 16], F32)
            nc.sync.dma_start(out=gsum_sb, in_=gsum.ap())
            gbc_sb = consts.tile([16, 128], F32)
            nc.sync.dma_start(out=gbc_sb, in_=gbc.ap())
            qkvwt_r = consts.tile([128, 2, 3 * C], F32R)
            projwt_r = consts.tile([128, 2, C], F32R)
            scale_sb = consts.tile([128, 2], F32)
            bias_sb = consts.tile([128, 2], F32)

            # x: [128, chunk, T]; first TH columns are this core's tokens
            x_sb = xpool.tile([128, 2, T], F32)
            for k in range(2):
                for q4 in range(4):
                    c0 = 1024 * q4
                    nc.sync.dma_start(out=x_sb[:, k, c0:c0 + 1024],
                                      in_=xb.ap()[k][:, c0:c0 + 1024])

            # persistent qkv outputs
            k_sb = kqv.tile([128, 2, T], F32R)
            q_sb = kqv.tile([128, 2, TH], F32R)
            vt_sb = kqv.tile([128, T // 128, NH, CH + 1], F32R)
            ones_st = consts.tile([128, T // 128, 1], F32)
            nc.vector.memset(ones_st, 1.0)
            for h in range(NH):
                nc.vector.tensor_copy(out=vt_sb[:, :, h, CH:CH + 1], in_=ones_st)
            ones_row_f = consts.tile([1, CH], F32)
            nc.vector.memset(ones_row_f, 1.0)
            ones_row = consts.tile([1, CH], F32R)
            nc.vector.tensor_copy(out=ones_row, in_=ones_row_f)

            with (
                tc.tile_pool(name="stage", bufs=1) as stage,
                tc.tile_pool(name="psmall", bufs=1, space="PSUM") as psmall,
            ):
                qkvwt_f = stage.tile([128, 2, 3 * C], F32)
                projwt_f = stage.tile([128, 2, C], F32)
                for k in range(2):
                    nc.sync.dma_start(out=qkvwt_f[:, k, :], in_=qkvwt.ap()[k])
                    nc.sync.dma_start(out=projwt_f[:, k, :], in_=projwt.ap()[k])
                nc.vector.tensor_copy(out=qkvwt_r, in_=qkvwt_f)
                nc.vector.tensor_copy(out=projwt_r, in_=projwt_f)

                # ---- groupnorm statistics ----
                # per-channel (mean, m2) -> per-group via PE (gsum), rstd,
                # then broadcast groups back to channels via PE (gbc).
                stat = stage.tile([128, 2, 2], F32)  # (mean_c, m2_c) per chunk
                sq = stage.tile([128, 1], F32)
                for k in range(2):
                    st6 = stage.tile([128, 8, 6], F32, bufs=2)
                    for sub in range(8):
                        nc.vector.bn_stats(out=st6[:, sub, :],
                                           in_=x_sb[:, k, 512 * sub:512 * (sub + 1)])
                    nc.vector.bn_aggr(out=stat[:, k, :], in_=st6)
                    # m2 = var + mean^2
                    nc.vector.tensor_tensor(out=sq, in0=stat[:, k, 0:1],
                                            in1=stat[:, k, 0:1], op=OP.mult)
                    nc.vector.tensor_tensor(out=stat[:, k, 1:2], in0=stat[:, k, 1:2],
                                            in1=sq, op=OP.add)
                pgrp = psmall.tile([16, 4], F32)
                nc.tensor.matmul(pgrp, gsum_sb,
                                 stat.rearrange("p a b -> p (a b)"),
                                 start=True, stop=True)
                # pgrp columns: (chunk k, val v) at 2k+v; v=0 mean, v=1 m2
                pgrp_kv = pgrp.rearrange("g (k v) -> g v k", v=2)
                meang = stage.tile([16, 2], F32)
                nc.vector.tensor_copy(out=meang, in_=pgrp_kv[:, 0, :])
                sqg = stage.tile([16, 2], F32)
                nc.vector.tensor_tensor(out=sqg, in0=meang, in1=meang, op=OP.mult)
                rstdg = stage.tile([16, 2], F32)
                nc.vector.tensor_tensor(out=rstdg, in0=pgrp_kv[:, 1, :], in1=sqg,
                                        op=OP.subtract)
                eps_t = stage.tile([16, 1], F32)
                nc.vector.memset(eps_t, EPS)
                nc.scalar.activation(out=rstdg, in_=rstdg, func=AF.Sqrt, bias=eps_t)
                nc.vector.reciprocal(out=rstdg, in_=rstdg)
                pm = psmall.tile([128, 2], F32)
                nc.tensor.matmul(pm, gbc_sb, meang, start=True, stop=True)
                pr = psmall.tile([128, 2], F32)
                nc.tensor.matmul(pr, gbc_sb, rstdg, start=True, stop=True)
                nc.vector.tensor_tensor(out=scale_sb, in0=pr, in1=normw_sb,
                                        op=OP.mult)
                nc.vector.tensor_tensor(out=bias_sb, in0=pm, in1=scale_sb,
                                        op=OP.mult)
                nc.vector.tensor_tensor(out=bias_sb, in0=normb_sb, in1=bias_sb,
                                        op=OP.subtract)

            # ---- qkv projection, streamed over T in 512-col tiles ----
            with (
                tc.tile_pool(name="xn", bufs=3) as xnp,
                tc.tile_pool(name="psqk", bufs=3, space="PSUM") as psqk,
                tc.tile_pool(name="psv", bufs=3, space="PSUM") as psv,
            ):
                for t8 in range(8):
                    t0 = 512 * t8
                    xn_t = xnp.tile([128, 2, 512], F32R)
                    for k in range(2):
                        nc.vector.tensor_scalar(
                            out=xn_t[:, k, :], in0=x_sb[:, k, t0:t0 + 512],
                            scalar1=scale_sb[:, k:k + 1],
                            scalar2=bias_sb[:, k:k + 1],
                            op0=OP.mult, op1=OP.add)
                    # k tiles (full T), q tiles (first TH only)
                    for j in range(2):
                        pk = psqk.tile([128, 512], F32, tag="qk")
                        for k in range(2):
                            nc.tensor.matmul(
                                pk, qkvwt_r[:, k, C + 128 * j:C + 128 * (j + 1)],
                                xn_t[:, k, :], start=(k == 0), stop=(k == 1))
                        nc.vector.tensor_scalar_add(
                            out=k_sb[:, j, t0:t0 + 512], in0=pk,
                            scalar1=qkvb_sb[:, 2 + j:3 + j])
                        if t8 < 4:
                            pq = psqk.tile([128, 512], F32, tag="qk")
                            for k in range(2):
                                nc.tensor.matmul(
                                    pq, qkvwt_r[:, k, 128 * j:128 * (j + 1)],
                                    xn_t[:, k, :], start=(k == 0), stop=(k == 1))
                            nc.vector.tensor_scalar_add(
                                out=q_sb[:, j, t0:t0 + 512], in0=pq,
                                scalar1=qkvb_sb[:, j:j + 1])
                    # v^T tiles
                    for u in range(4):
                        pv = psv.tile([128, C], F32)
                        for k in range(2):
                            nc.tensor.matmul(
                                pv, xn_t[:, k, 128 * u:128 * (u + 1)],
                                qkvwt_r[:, k, 2 * C:3 * C],
                                start=(k == 0), stop=(k == 1))
                        nc.vector.tensor_copy(
                            out=vt_sb[:, 4 * t8 + u, :, 0:CH], in_=pv)

            # ---- attention + projection ----
            with (
                tc.tile_pool(name="apool", bufs=1) as apool,
                tc.tile_pool(name="wexp", bufs=8) as wexp,
                tc.tile_pool(name="rpool", bufs=2) as rpool,
                tc.tile_pool(name="opool", bufs=1) as opool,
                tc.tile_pool(name="pss", bufs=2, space="PSUM") as pss,
                tc.tile_pool(name="psa", bufs=2, space="PSUM") as psa,
                tc.tile_pool(name="psh", bufs=2, space="PSUM") as psh,
            ):
                a_sb = apool.tile([128, 2, TH], F32R)
                out_sb = opool.tile([128, 2, TH], F32)
                n_p = T // 256  # 16 paired score chunks (2 x 128 rows each)
                for ti in range(4):
                    t0 = 512 * ti
                    for h in range(NH):
                        p0 = CH * (h % 2)
                        j = h // 2
                        pa = psa.tile([CH + 1, 512], F32)
                        # software-pipelined: scores(i+1) issued before av(i)
                        w_l = [None, None]
                        for s in range(n_p + 1):
                            if s < n_p:
                                # scores for chunks 2s, 2s+1 -> one psum pair
                                ps = pss.tile([128, 2, 512], F32)
                                for u in range(2):
                                    nc.tensor.matmul(
                                        ps[:, u, :],
                                        k_sb[p0:p0 + CH, j,
                                             128 * (2 * s + u):128 * (2 * s + u + 1)],
                                        q_sb[p0:p0 + CH, j, t0:t0 + 512],
                                        start=True, stop=True)
                                w_t = wexp.tile([128, 2, 512], F32R)
                                nc.scalar.activation(out=w_t, in_=ps, func=AF.Exp,
                                                     scale=SCALE2)
                                w_l[s % 2] = w_t
                            if s > 0:
                                for u in range(2):
                                    sc = 2 * (s - 1) + u
                                    nc.tensor.matmul(
                                        pa, vt_sb[:, sc, h, :],
                                        w_l[(s - 1) % 2][:, u, :],
                                        start=(sc == 0), stop=(sc == 2 * n_p - 1))
                        recip = rpool.tile([1, 512], F32)
                        nc.vector.reciprocal(out=recip, in_=pa[CH:CH + 1, :])
                        recip_r = rpool.tile([1, 512], F32R)
                        nc.vector.tensor_copy(out=recip_r, in_=recip)
                        rb = psh.tile([128, 512], F32, tag="ph")
                        nc.tensor.matmul(rb[0:CH, :], ones_row, recip_r,
                                         start=True, stop=True)
                        a_sl = a_sb[p0:p0 + CH, j, t0:t0 + 512]
                        nc.vector.tensor_copy(out=a_sl, in_=pa[0:CH, :])
                        nc.vector.tensor_tensor(out=a_sl, in0=a_sl,
                                                in1=rb[0:CH, :], op=OP.mult)
                        nc.vector.tensor_scalar_add(
                            out=a_sl, in0=a_sl,
                            scalar1=qkvb_sb[p0:p0 + CH, 4 + j:5 + j])
                    # proj for this t-tile (all heads of column range ready)
                    for j in range(2):
                        ph = psh.tile([128, 512], F32)
                        for k in range(2):
                            nc.tensor.matmul(
                                ph, projwt_r[:, k, 128 * j:128 * (j + 1)],
                                a_sb[:, k, t0:t0 + 512], start=(k == 0),
                                stop=(k == 1))
                        o_sl = out_sb[:, j, t0:t0 + 512]
                        nc.vector.tensor_tensor(out=o_sl, in0=ph,
                                                in1=x_sb[:, j, t0:t0 + 512],
                                                op=OP.add)
                        nc.vector.tensor_scalar_add(
                            out=o_sl, in0=o_sl, scalar1=projb_sb[:, j:j + 1])
                        nc.sync.dma_start(out=out_d.ap()[j, :, t0:t0 + 512],
                                          in_=o_sl)

    split_excess_waits(nc)
    return nc


_NC_CACHE = {}


def _get_nc(repeat=1):
    if repeat not in _NC_CACHE:
        _NC_CACHE[repeat] = build_nc(repeat)
    return _NC_CACHE[repeat]


def _shard_inputs(x, norm_w, norm_b, qkv_w, qkv_b, proj_w, proj_b):
    xr = np.ascontiguousarray(x.reshape(B, 2, 128, T).astype(np.float32))
    # Reference splits qkv head-blockwise: head h uses rows [192h, 192h+192)
    # as (q|k|v). Permute rows to our layout: q all heads head-major, then k,
    # then v.
    perm = np.concatenate([
        np.concatenate([np.arange(3 * CH * h + CH * p, 3 * CH * h + CH * (p + 1))
                        for h in range(NH)])
        for p in range(3)])
    qkv_w = np.asarray(qkv_w)[perm]
    qkv_b = np.asarray(qkv_b)[perm]
    qkvwt = np.ascontiguousarray(qkv_w.T.reshape(2, 128, 3 * C).astype(np.float32))
    projwt = np.ascontiguousarray(proj_w.T.reshape(2, 128, C).astype(np.float32))
    qkvb = np.ascontiguousarray(qkv_b.reshape(6, 128).T.astype(np.float32))
    projb = np.ascontiguousarray(proj_b.reshape(2, 128).T.astype(np.float32))
    normw = np.ascontiguousarray(norm_w.reshape(2, 128).T.astype(np.float32))
    normb = np.ascontiguousarray(norm_b.reshape(2, 128).T.astype(np.float32))
    p = np.arange(128)
    gsum = (p[:, None] // 8 == np.arange(16)[None, :]).astype(np.float32) / GS
    gbc = (np.arange(16)[:, None] == p[None, :] // 8).astype(np.float32)

    in_maps = []
    for c in range(N_CORES):
        b, half = c // 2, c % 2
        # roll T so this core's tokens are the first TH columns
        xc = np.roll(xr[b], -half * TH, axis=2) if half else xr[b]
        in_maps.append({
            "xb": np.ascontiguousarray(xc),
            "qkvwt": qkvwt, "projwt": projwt,
            "qkvb": qkvb, "projb": projb,
            "normw": normw, "normb": normb,
            "gsum": gsum, "gbc": gbc,
        })
    return in_maps


def _assemble(results):
    out = np.empty((B, 2, 128, T), np.float32)
    for c in range(N_CORES):
        b, half = c // 2, c % 2
        out[b, :, :, half * TH:(half + 1) * TH] = results[c]["out"]
    return out.reshape(B, C, HH, WW)


def kernel(x, norm_w, norm_b, qkv_w, qkv_b, proj_w, proj_b):
    nc = _get_nc()
    in_maps = _shard_inputs(x, norm_w, norm_b, qkv_w, qkv_b, proj_w, proj_b)
    res = run_bass_kernel_spmd(nc, in_maps, core_ids=list(range(N_CORES)))
    return _assemble(res.results)
